# revision 7
# baseline (speedup 1.0000x reference)
"""TRN2 Bass kernel: 2-bit-quantized linear  y = x @ (levels[idx] * scale).T + bias.

Sharding: column-parallel over 8 NeuronCores - each core owns OUT_F/8 output
features (its slice of the weights / scales / bias); x is replicated.

Design: the 2-bit weights are dequantized to fp16 on the HOST (levels[idx] is
the same byte count in fp16 as the fp16-encoded indices, so DMA traffic is
unchanged and the device does zero dequant work).  The per-output scale and
bias are folded into the PSUM drain (one ScalarE activation per output tile).

Per-core device algorithm:
  - W^T [IN_F, O_SHARD] fp16 is DMA'd once, k-tile by k-tile, and cached in
    SBUF (96 KiB/partition).
  - x^T is streamed in [128, 512]-token chunks, double-buffered.
  - Chunk 0 runs k-outer across 8 PSUM banks (8 o-tiles accumulate in
    lockstep) so the PE consumes each W k-tile the moment its DMA lands;
    remaining o-tiles and all later chunks run k-inner per o-tile with all
    weights resident.
  - PSUM drain fuses scale and bias via one ScalarE activation with
    per-partition scale/bias vectors, giving y^T fp32 directly.

The host transposes x / W on the way in and y^T on the way out; those are
layout moves only.
"""

import numpy as np
import ml_dtypes

import concourse.bass as bass
import concourse.bacc as bacc
import concourse.tile as tile
import concourse.mybir as mybir
from concourse.bass_utils import run_bass_kernel_spmd

AF = mybir.ActivationFunctionType
DT = mybir.dt

NCORES = 8

# Problem sizes (hardcoded per contract).
B, S, IN_F, OUT_F = 4, 1024, 4096, 12288
T_TOKENS = B * S
O_SHARD = OUT_F // NCORES


def build_program(
    *,
    in_f: int,
    t_tokens: int,
    o_shard: int,
    tc_size: int = 512,
    x_extra_bufs: int = 32,
    out_bufs: int = 6,
    ramp_banks: int = 8,
    warmup_mms: int = 2,
):
    """Build the single-core Bass/Tile program (SPMD across cores)."""
    assert in_f % 128 == 0 and o_shard % 128 == 0 and t_tokens % tc_size == 0
    kt = in_f // 128
    n_ot = o_shard // 128
    n_tc = t_tokens // tc_size

    nc = bacc.Bacc("TRN2", target_bir_lowering=False, debug=False)

    xt_d = nc.dram_tensor("xt", [in_f, t_tokens], DT.float16, kind="ExternalInput")
    wt_d = nc.dram_tensor("wt", [in_f, o_shard], DT.float8e3, kind="ExternalInput")
    scl_d = nc.dram_tensor("scl", [128, n_ot], DT.float32, kind="ExternalInput")
    bsv_d = nc.dram_tensor("bsv", [128, n_ot], DT.float32, kind="ExternalInput")
    yt_d = nc.dram_tensor("yt", [o_shard, t_tokens], DT.float16, kind="ExternalOutput")

    with tile.TileContext(nc) as tc:
        with (
            tc.tile_pool(name="const", bufs=1) as cpool,
            tc.tile_pool(name="wt", bufs=kt) as wtp,
            tc.tile_pool(name="xtp", bufs=kt + x_extra_bufs) as xtp,
            tc.tile_pool(name="outp", bufs=out_bufs) as outp,
            tc.tile_pool(name="ps", bufs=8, space=bass.MemorySpace.PSUM) as psp,
        ):
            # PE warmup: a few dummy matmuls on a zeroed tile start the HAM
            # activity window while the first DMAs land, so the real matmul
            # stream reaches the warm 2.4 GHz clock sooner.
            if warmup_mms:
                warm_t = cpool.tile([128, tc_size], DT.float16, tag="warm")
                nc.gpsimd.memset(warm_t[:], 0.0)
                warm_ps = psp.tile([128, tc_size], DT.float32, tag="ps", name="warm")
                for _ in range(warmup_mms):
                    nc.tensor.matmul(
                        warm_ps[:], warm_t[:, 0:128], warm_t[:], start=True, stop=True
                    )

            # Tiny constant loads go first on each HW DGE queue: they warm
            # the DMA path so the first W / x transfers run at full rate.
            bsv_t = cpool.tile([128, n_ot], DT.float32, tag="bsv")
            nc.sync.dma_start(bsv_t[:], bsv_d[:])
            scl_t = cpool.tile([128, n_ot], DT.float32, tag="scl")
            nc.scalar.dma_start(scl_t[:], scl_d[:])

            # W k-tile loads on the sync queue; x streams on the vector queue
            # so the two transfer concurrently during the ramp.
            wts = []
            for k in range(kt):
                w = wtp.tile([128, o_shard], DT.float8e3, tag="wt")
                nc.sync.dma_start(w[:], wt_d[k * 128 : (k + 1) * 128, :])
                wts.append(w)

            def load_chunk(tci):
                eng = nc.scalar if tci == 0 else nc.sync
                xts = []
                for k in range(kt):
                    xt_t = xtp.tile([128, tc_size], DT.float16, tag="xt")
                    eng.dma_start(
                        xt_t[:],
                        xt_d[
                            k * 128 : (k + 1) * 128,
                            tci * tc_size : (tci + 1) * tc_size,
                        ],
                    )
                    xts.append(xt_t)
                return xts

            def drain_store(ps, ot, tci, t0=0, tn=None):
                tn = tc_size if tn is None else tn
                out_t = outp.tile([128, tn], DT.float16, tag="out")
                nc.scalar.activation(
                    out_t[:],
                    ps[:],
                    AF.Identity,
                    bias=bsv_t[:, ot : ot + 1],
                    scale=scl_t[:, ot : ot + 1],
                )
                nc.scalar.dma_start(
                    yt_d[
                        ot * 128 : (ot + 1) * 128,
                        tci * tc_size + t0 : tci * tc_size + t0 + tn,
                    ],
                    out_t[:],
                )

            def mm_group(xts, ot, tci, t0=0, tn=None):
                tn = tc_size if tn is None else tn
                ps = psp.tile([128, tn], DT.float32, tag="ps")
                for k in range(kt):
                    nc.tensor.matmul(
                        ps[:],
                        wts[k][:, ot * 128 : (ot + 1) * 128],
                        xts[k][:, t0 : t0 + tn],
                        start=(k == 0),
                        stop=(k == kt - 1),
                    )
                drain_store(ps, ot, tci, t0, tn)

            for tci in range(n_tc):
                xts = load_chunk(tci)
                if tci == 0 and ramp_banks:
                    # k-outer across `ramp_banks` PSUM banks: the PE consumes
                    # each (W, x) k-tile pair the moment the DMA delivers it.
                    ra = list(range(min(ramp_banks, n_ot)))
                    pss = {
                        ot: psp.tile([128, tc_size], DT.float32, tag="ps", name="ps")
                        for ot in ra
                    }
                    for k in range(kt):
                        for ot in ra:
                            nc.tensor.matmul(
                                pss[ot][:],
                                wts[k][:, ot * 128 : (ot + 1) * 128],
                                xts[k][:],
                                start=(k == 0),
                                stop=(k == kt - 1),
                            )
                    for ot in ra:
                        drain_store(pss[ot], ot, tci)
                    rest = range(len(ra), n_ot)
                else:
                    rest = range(n_ot)
                for ot in rest:
                    if tci == n_tc - 1 and ot == n_ot - 1 and tc_size >= 256:
                        h = tc_size // 2
                        mm_group(xts, ot, tci, 0, h)
                        mm_group(xts, ot, tci, h, h)
                    else:
                        mm_group(xts, ot, tci)

    nc.compile()
    return nc


def quant_levels(levels):
    """Quantize the 4 levels to fp8 e3m4 with the best global pre-scale s
    (folded back into the per-output drain scale)."""
    lv = np.asarray(levels, dtype=np.float64)
    best = (np.inf, 1.0)
    for sc in np.geomspace(0.25, 4.0, 257):
        q = np.asarray(lv / sc, dtype=np.float32).astype(
            ml_dtypes.float8_e3m4).astype(np.float64) * sc
        r = float(np.sqrt(np.mean((q - lv) ** 2)))
        if r < best[0]:
            best = (r, float(sc))
    sc = best[1]
    lut = np.asarray(lv / sc, dtype=np.float32).astype(ml_dtypes.float8_e3m4)
    return lut, sc


def make_in_maps(x, levels, weight_indices, weight_scales, bias):
    """Host-side shard + layout prep: one input map per core."""
    t_tokens = x.shape[0] * x.shape[1]
    in_f = x.shape[2]
    o_shard = weight_indices.shape[0] // NCORES
    n_ot = o_shard // 128

    x2 = np.asarray(x, dtype=np.float32).reshape(t_tokens, in_f)
    xt = np.ascontiguousarray(x2.T).astype(np.float16)

    # Host dequant: W^T[k, o] = e3m4(levels/s)[idx^T[k, o]]; s and the
    # per-output scale are folded into the drain.
    lut, lv_scale = quant_levels(levels)
    wt_full = lut[np.asarray(weight_indices).T]  # [IN_F, OUT_F] fp8e3

    in_maps = []
    for c in range(NCORES):
        o0, o1 = c * o_shard, (c + 1) * o_shard
        wt = np.ascontiguousarray(wt_full[:, o0:o1])
        scl = np.ascontiguousarray(
            (np.asarray(weight_scales[o0:o1], dtype=np.float64) * lv_scale)
            .astype(np.float32).reshape(n_ot, 128).T
        )
        bsv = np.ascontiguousarray(
            np.asarray(bias[o0:o1], dtype=np.float32).reshape(n_ot, 128).T
        )
        in_maps.append({"xt": xt, "wt": wt, "scl": scl, "bsv": bsv})
    return in_maps


_PROGRAM_CACHE: dict = {}


def _get_program():
    if "p" not in _PROGRAM_CACHE:
        _PROGRAM_CACHE["p"] = build_program(
            in_f=IN_F, t_tokens=T_TOKENS, o_shard=O_SHARD
        )
    return _PROGRAM_CACHE["p"]


def run_on_cores(x, levels, weight_indices, weight_scales, bias, *, trace=False):
    nc = _get_program()
    in_maps = make_in_maps(x, levels, weight_indices, weight_scales, bias)
    res = run_bass_kernel_spmd(
        nc, in_maps, core_ids=list(range(NCORES)), trace=trace
    )
    yt = np.concatenate([res.results[c]["yt"] for c in range(NCORES)], axis=0)
    y = np.ascontiguousarray(yt.T).astype(np.float32).reshape(B, S, OUT_F)
    return y, res


def kernel(x, levels, weight_indices, weight_scales, bias):
    y, _ = run_on_cores(x, levels, weight_indices, weight_scales, bias)
    return y


# revision 8
# speedup vs baseline: 1.0054x; 1.0054x over previous
"""TRN2 Bass kernel: 2-bit-quantized linear  y = x @ (levels[idx] * scale).T + bias.

Sharding: column-parallel over 8 NeuronCores - each core owns OUT_F/8 output
features (its slice of the weights / scales / bias); x is replicated.

Design: the 2-bit weights are dequantized to fp16 on the HOST (levels[idx] is
the same byte count in fp16 as the fp16-encoded indices, so DMA traffic is
unchanged and the device does zero dequant work).  The per-output scale and
bias are folded into the PSUM drain (one ScalarE activation per output tile).

Per-core device algorithm:
  - W^T [IN_F, O_SHARD] fp16 is DMA'd once, k-tile by k-tile, and cached in
    SBUF (96 KiB/partition).
  - x^T is streamed in [128, 512]-token chunks, double-buffered.
  - Chunk 0 runs k-outer across 8 PSUM banks (8 o-tiles accumulate in
    lockstep) so the PE consumes each W k-tile the moment its DMA lands;
    remaining o-tiles and all later chunks run k-inner per o-tile with all
    weights resident.
  - PSUM drain fuses scale and bias via one ScalarE activation with
    per-partition scale/bias vectors, giving y^T fp32 directly.

The host transposes x / W on the way in and y^T on the way out; those are
layout moves only.
"""

import numpy as np
import ml_dtypes

import concourse.bass as bass
import concourse.bacc as bacc
import concourse.tile as tile
import concourse.mybir as mybir
from concourse.bass_utils import run_bass_kernel_spmd

AF = mybir.ActivationFunctionType
DT = mybir.dt

NCORES = 8

# Problem sizes (hardcoded per contract).
B, S, IN_F, OUT_F = 4, 1024, 4096, 12288
T_TOKENS = B * S
O_SHARD = OUT_F // NCORES


def build_program(
    *,
    in_f: int,
    t_tokens: int,
    o_shard: int,
    tc_size: int = 512,
    x_extra_bufs: int = 32,
    out_bufs: int = 6,
    ramp_banks: int = 8,
    warmup_mms: int = 32,
):
    """Build the single-core Bass/Tile program (SPMD across cores)."""
    assert in_f % 128 == 0 and o_shard % 128 == 0 and t_tokens % tc_size == 0
    kt = in_f // 128
    n_ot = o_shard // 128
    n_tc = t_tokens // tc_size

    nc = bacc.Bacc("TRN2", target_bir_lowering=False, debug=False)

    xt_d = nc.dram_tensor("xt", [in_f, t_tokens], DT.float16, kind="ExternalInput")
    wt_d = nc.dram_tensor("wt", [in_f, o_shard], DT.float8e3, kind="ExternalInput")
    scl_d = nc.dram_tensor("scl", [128, n_ot], DT.float32, kind="ExternalInput")
    bsv_d = nc.dram_tensor("bsv", [128, n_ot], DT.float32, kind="ExternalInput")
    yt_d = nc.dram_tensor("yt", [o_shard, t_tokens], DT.float16, kind="ExternalOutput")

    with tile.TileContext(nc) as tc:
        with (
            tc.tile_pool(name="const", bufs=1) as cpool,
            tc.tile_pool(name="wt", bufs=kt) as wtp,
            tc.tile_pool(name="xtp", bufs=kt + x_extra_bufs) as xtp,
            tc.tile_pool(name="outp", bufs=out_bufs) as outp,
            tc.tile_pool(name="ps", bufs=8, space=bass.MemorySpace.PSUM) as psp,
        ):
            # PE warmup: a few dummy matmuls on a zeroed tile start the HAM
            # activity window while the first DMAs land, so the real matmul
            # stream reaches the warm 2.4 GHz clock sooner.
            if warmup_mms:
                warm_t = cpool.tile([128, 128], DT.float16, tag="warm")
                nc.gpsimd.memset(warm_t[:], 0.0)
                warm_ps = psp.tile([128, 128], DT.float32, tag="ps", name="warm")
                for _ in range(warmup_mms):
                    nc.tensor.matmul(
                        warm_ps[:], warm_t[:], warm_t[:], start=True, stop=True
                    )

            # Tiny constant loads go first on each HW DGE queue: they warm
            # the DMA path so the first W / x transfers run at full rate.
            bsv_t = cpool.tile([128, n_ot], DT.float32, tag="bsv")
            nc.sync.dma_start(bsv_t[:], bsv_d[:])
            scl_t = cpool.tile([128, n_ot], DT.float32, tag="scl")
            nc.scalar.dma_start(scl_t[:], scl_d[:])

            # W k-tile loads on the sync queue; x streams on the vector queue
            # so the two transfer concurrently during the ramp.
            wts = []
            for k in range(kt):
                w = wtp.tile([128, o_shard], DT.float8e3, tag="wt")
                nc.sync.dma_start(w[:], wt_d[k * 128 : (k + 1) * 128, :])
                wts.append(w)

            def load_chunk(tci):
                eng = nc.scalar if tci == 0 else nc.sync
                xts = []
                for k in range(kt):
                    xt_t = xtp.tile([128, tc_size], DT.float16, tag="xt")
                    eng.dma_start(
                        xt_t[:],
                        xt_d[
                            k * 128 : (k + 1) * 128,
                            tci * tc_size : (tci + 1) * tc_size,
                        ],
                    )
                    xts.append(xt_t)
                return xts

            def drain_store(ps, ot, tci, t0=0, tn=None):
                tn = tc_size if tn is None else tn
                out_t = outp.tile([128, tn], DT.float16, tag="out")
                nc.scalar.activation(
                    out_t[:],
                    ps[:],
                    AF.Identity,
                    bias=bsv_t[:, ot : ot + 1],
                    scale=scl_t[:, ot : ot + 1],
                )
                nc.scalar.dma_start(
                    yt_d[
                        ot * 128 : (ot + 1) * 128,
                        tci * tc_size + t0 : tci * tc_size + t0 + tn,
                    ],
                    out_t[:],
                )

            def mm_group(xts, ot, tci, t0=0, tn=None):
                tn = tc_size if tn is None else tn
                ps = psp.tile([128, tn], DT.float32, tag="ps")
                for k in range(kt):
                    nc.tensor.matmul(
                        ps[:],
                        wts[k][:, ot * 128 : (ot + 1) * 128],
                        xts[k][:, t0 : t0 + tn],
                        start=(k == 0),
                        stop=(k == kt - 1),
                    )
                drain_store(ps, ot, tci, t0, tn)

            for tci in range(n_tc):
                xts = load_chunk(tci)
                if tci == 0 and ramp_banks:
                    # k-outer across `ramp_banks` PSUM banks: the PE consumes
                    # each (W, x) k-tile pair the moment the DMA delivers it.
                    ra = list(range(min(ramp_banks, n_ot)))
                    pss = {
                        ot: psp.tile([128, tc_size], DT.float32, tag="ps", name="ps")
                        for ot in ra
                    }
                    for k in range(kt):
                        for ot in ra:
                            nc.tensor.matmul(
                                pss[ot][:],
                                wts[k][:, ot * 128 : (ot + 1) * 128],
                                xts[k][:],
                                start=(k == 0),
                                stop=(k == kt - 1),
                            )
                    for ot in ra:
                        drain_store(pss[ot], ot, tci)
                    rest = range(len(ra), n_ot)
                else:
                    rest = range(n_ot)
                for ot in rest:
                    if tci == n_tc - 1 and ot == n_ot - 1 and tc_size >= 256:
                        h = tc_size // 2
                        mm_group(xts, ot, tci, 0, h)
                        mm_group(xts, ot, tci, h, h)
                    else:
                        mm_group(xts, ot, tci)

    nc.compile()
    return nc


def quant_levels(levels):
    """Quantize the 4 levels to fp8 e3m4 with the best global pre-scale s
    (folded back into the per-output drain scale)."""
    lv = np.asarray(levels, dtype=np.float64)
    best = (np.inf, 1.0)
    for sc in np.geomspace(0.25, 4.0, 257):
        q = np.asarray(lv / sc, dtype=np.float32).astype(
            ml_dtypes.float8_e3m4).astype(np.float64) * sc
        r = float(np.sqrt(np.mean((q - lv) ** 2)))
        if r < best[0]:
            best = (r, float(sc))
    sc = best[1]
    lut = np.asarray(lv / sc, dtype=np.float32).astype(ml_dtypes.float8_e3m4)
    return lut, sc


def make_in_maps(x, levels, weight_indices, weight_scales, bias):
    """Host-side shard + layout prep: one input map per core."""
    t_tokens = x.shape[0] * x.shape[1]
    in_f = x.shape[2]
    o_shard = weight_indices.shape[0] // NCORES
    n_ot = o_shard // 128

    x2 = np.asarray(x, dtype=np.float32).reshape(t_tokens, in_f)
    xt = np.ascontiguousarray(x2.T).astype(np.float16)

    # Host dequant: W^T[k, o] = e3m4(levels/s)[idx^T[k, o]]; s and the
    # per-output scale are folded into the drain.
    lut, lv_scale = quant_levels(levels)
    wt_full = lut[np.asarray(weight_indices).T]  # [IN_F, OUT_F] fp8e3

    in_maps = []
    for c in range(NCORES):
        o0, o1 = c * o_shard, (c + 1) * o_shard
        wt = np.ascontiguousarray(wt_full[:, o0:o1])
        scl = np.ascontiguousarray(
            (np.asarray(weight_scales[o0:o1], dtype=np.float64) * lv_scale)
            .astype(np.float32).reshape(n_ot, 128).T
        )
        bsv = np.ascontiguousarray(
            np.asarray(bias[o0:o1], dtype=np.float32).reshape(n_ot, 128).T
        )
        in_maps.append({"xt": xt, "wt": wt, "scl": scl, "bsv": bsv})
    return in_maps


_PROGRAM_CACHE: dict = {}


def _get_program():
    if "p" not in _PROGRAM_CACHE:
        _PROGRAM_CACHE["p"] = build_program(
            in_f=IN_F, t_tokens=T_TOKENS, o_shard=O_SHARD
        )
    return _PROGRAM_CACHE["p"]


def run_on_cores(x, levels, weight_indices, weight_scales, bias, *, trace=False):
    nc = _get_program()
    in_maps = make_in_maps(x, levels, weight_indices, weight_scales, bias)
    res = run_bass_kernel_spmd(
        nc, in_maps, core_ids=list(range(NCORES)), trace=trace
    )
    yt = np.concatenate([res.results[c]["yt"] for c in range(NCORES)], axis=0)
    y = np.ascontiguousarray(yt.T).astype(np.float32).reshape(B, S, OUT_F)
    return y, res


def kernel(x, levels, weight_indices, weight_scales, bias):
    y, _ = run_on_cores(x, levels, weight_indices, weight_scales, bias)
    return y
